# revision 27
# baseline (speedup 1.0000x reference)
"""Trainium2 Bass kernel for nn_MultiHeadAttention_46188078301212.

Module semantics (replicated from the PyTorch module's quirky reshape):
  P_q = q @ Wq.T + bq  (same for k, v), each [B, 2048, 512]
  Head h takes projection rows [256h, 256h+256) viewed as [2048, 64]
  (row-major), runs standard softmax attention, heads are concatenated
  along hidden (out col block 64h..64h+64) and merged with Wm.T + bm.

Sharding: 8 cores = (batch b in {0,1}) x (head-pair j4 in {0..3}).
Core c handles b = c//4 and global heads {2*j4, 2*j4+1}, which only need
projection rows [512*j4, 512*j4+512) of q/k/v for that batch.  Each core
emits a partial [2048, 512] output (its 2 heads' contribution to the
merge matmul); the host sums 4 partials per batch and adds bm.

On-core layout notes:
  - Head-local index is permuted: q' = c*256 + s (c = hidden chunk e//64,
    s = local row).  Softmax is permutation-invariant; the final DMA
    un-permutes rows back to the true order.
  - Scores are computed transposed (S^T: keys on partitions, queries on
    free axis) so that P@V can contract over keys on the partition axis.
  - Attention runs in two q'-half phases; within a phase, kappa'-tile
    pairs (j, j+8) are row-packed onto array rows 0:63 / 64:127 so two
    K=64 matmuls share the PE array.
  - Softmax denominators come from an all-ones column appended to V.
  - All matmuls are bf16 (psum accumulation f32); probs are bf16.
"""

import numpy as np

HIDDEN = 512
DHEAD = 64
B = 2
S = 2048
NCORES = 8

_CACHE = {}

# Filled in by the last kernel() call when BASS_TRACE is set.
LAST_EXEC_NS = None
LAST_RESULTS = None


def _build_nc():
    if "nc" in _CACHE:
        return _CACHE["nc"]

    import contextlib

    import concourse.mybir as mybir
    import concourse.tile as tile
    from concourse import bacc

    f32 = mybir.dt.float32
    bf16 = mybir.dt.bfloat16
    Exp = mybir.ActivationFunctionType.Exp
    Copy = mybir.ActivationFunctionType.Copy
    mult = mybir.AluOpType.mult
    add = mybir.AluOpType.add

    nc = bacc.Bacc("TRN2", target_bir_lowering=False)

    # ---- DRAM I/O ----
    d_x = {}
    d_w = {}
    for nm in ("q", "k", "v"):
        d_x[nm] = nc.dram_tensor(f"x{nm}T", [512, 512], bf16, kind="ExternalInput")
        wcols = 1024 if nm == "q" else 512
        d_w[nm] = nc.dram_tensor(
            f"w{nm}T", [512, wcols], bf16, kind="ExternalInput"
        )
    d_wm = [
        nc.dram_tensor("wm0", [128, 512], bf16, kind="ExternalInput"),
        nc.dram_tensor("wm1", [128, 512], bf16, kind="ExternalInput"),
    ]
    d_wu = nc.dram_tensor("wup", [128, 512], bf16, kind="ExternalInput")
    d_bq = nc.dram_tensor("bq8", [128, 8], f32, kind="ExternalInput")
    d_bk = nc.dram_tensor("bk4", [128, 4], f32, kind="ExternalInput")
    d_bv = nc.dram_tensor("bv1", [1, 512], bf16, kind="ExternalInput")
    d_on = nc.dram_tensor("ones1", [1, 128], bf16, kind="ExternalInput")
    d_out = nc.dram_tensor("outp", [2048, 512], f32, kind="ExternalOutput")
    d_out_r = d_out.rearrange("(b p a) e -> p b a e", b=2, p=128)

    with tile.TileContext(nc) as tc:
        ctx = contextlib.ExitStack()
        with ctx:
            big = ctx.enter_context(tc.tile_pool(name="big", bufs=8))
            sp = ctx.enter_context(tc.tile_pool(name="sp", bufs=2, space="PSUM"))
            op = ctx.enter_context(tc.tile_pool(name="op", bufs=2, space="PSUM"))
            pp = ctx.enter_context(tc.tile_pool(name="pp", bufs=2, space="PSUM"))
            ptp = ctx.enter_context(tc.tile_pool(name="ptp", bufs=12))
            small = ctx.enter_context(tc.tile_pool(name="small", bufs=1))
            otp = ctx.enter_context(tc.tile_pool(name="otp", bufs=2))
            drp = ctx.enter_context(tc.tile_pool(name="drp", bufs=2))
            rcp = ctx.enter_context(tc.tile_pool(name="rcp", bufs=2))
            tmpp = ctx.enter_context(tc.tile_pool(name="tmpp", bufs=2))

            # ---- constants / small tensors ----
            ones_tiny = small.tile([1, 1], f32, tag="ones_tiny")
            nc.vector.memset(ones_tiny, 1.0)
            warm = small.tile([1, 1], f32, tag="warm")
            nc.scalar.activation(warm, ones_tiny, Exp, scale=1.0)

            # ---- warmup weights + bias first, then big inputs/weights ----
            wup = small.tile([128, 512], bf16, tag="wup")
            nc.sync.dma_start(out=wup, in_=d_wu[:, :])
            bq8 = small.tile([128, 8], f32, tag="bq8")
            nc.sync.dma_start(out=bq8, in_=d_bq[:, :])


            xin = {}
            win = {}
            for nm in ("q", "k", "v"):
                eng = nc.sync if nm == "q" else nc.scalar
                xt = big.tile([128, 4, 512], bf16, tag="big", name=f"x{nm}")
                eng.dma_start(
                    out=xt, in_=d_x[nm].rearrange("(t p) d -> p t d", p=128)
                )
                wcols = 1024 if nm == "q" else 512
                wt = big.tile([128, 4, wcols], bf16, tag="big", name=f"w{nm}")
                eng.dma_start(
                    out=wt, in_=d_w[nm].rearrange("(t p) d -> p t d", p=128)
                )
                xin[nm] = xt
                win[nm] = wt

            bk4 = small.tile([128, 4], f32, tag="bk4")
            nc.sync.dma_start(out=bk4, in_=d_bk[:, :])
            bv1 = small.tile([1, 512], bf16, tag="bv1")
            nc.sync.dma_start(out=bv1, in_=d_bv[:, :])
            ones1 = small.tile([1, 128], bf16, tag="ones1")
            nc.sync.dma_start(out=ones1, in_=d_on[:, :])
            wm_sb = []
            for t in range(2):
                w = small.tile([128, 512], bf16, tag=f"wm{t}", name=f"wm{t}")
                nc.sync.dma_start(out=w, in_=d_wm[t][:, :])
                wm_sb.append(w)

            # ---- per-head working tensors ----
            # QT[t]: [128, 2048]  rows 0:64 = Q^T full q' (chunk c at cols
            #                     256c); rows 64:128 = identical duplicate
            # KT[t]: [128, 1024]  rows 0:64 = K^T kappa' [0,1024) (chunk c at
            #                     cols 256c); rows 64:128 = kappa' [1024,2048)
            # V[t]:  [128, 16, 65] bf16; [:, j, 0:64] = V chunk j, col 64 = 1
            OTb = [
                small.tile([128, 1024], bf16, tag=f"OTb{i}", name=f"OTb{i}")
                for i in range(2)
            ]
            for i in range(2):
                nc.vector.memset(OTb[i][64:128, :], 0.0)
            QLO = small.tile([128, 2, 2048], bf16, tag="QLO", name="QLO")
            QHI = small.tile([128, 2, 2048], bf16, tag="QHI", name="QHI")
            nc.vector.memset(QLO[64:128, :, :], 0.0)
            nc.vector.memset(QHI[0:64, :, :], 0.0)
            KT2 = small.tile([128, 2, 1024], bf16, tag="KT2", name="KT2")
            Vt = [
                small.tile([128, 16, 65], bf16, tag=f"V{t}", name=f"V{t}")
                for t in range(2)
            ]
            for t in range(2):
                nc.vector.memset(Vt[t][:, :, 64:65], 1.0)

            # ---- projection emitters (closures; some upfront, rest
            # spread into the attention stream to overlap with it) ----
            def proj_q(cq):
                xt, wt = xin["q"], win["q"]
                ps = pp.tile([128, 512], f32, tag="pp", name="psq")
                for D in range(4):
                    nc.tensor.matmul(
                        ps,
                        wt[:, D, 128 * cq : 128 * cq + 128],
                        xt[:, D, :],
                        start=(D == 0),
                        stop=(D == 3),
                    )
                psr = ps.rearrange("p (t s) -> p t s", t=2)
                nc.vector.tensor_scalar_add(
                    QLO[0:64, :, 256 * cq : 256 * cq + 256],
                    psr[0:64, :, :],
                    bq8[0:64, cq : cq + 1],
                )
                nc.vector.tensor_scalar_add(
                    QHI[64:128, :, 256 * cq : 256 * cq + 256],
                    psr[64:128, :, :],
                    bq8[64:128, cq : cq + 1],
                )

            def proj_k(cp):
                xt, wt = xin["k"], win["k"]
                ps = pp.tile([128, 512], f32, tag="pp", name="psk")
                for D in range(4):
                    nc.tensor.matmul(
                        ps,
                        wt[:, D, 128 * cp : 128 * cp + 128],
                        xt[:, D, :],
                        start=(D == 0),
                        stop=(D == 3),
                    )
                nc.vector.tensor_scalar_add(
                    KT2[:, :, 256 * cp : 256 * cp + 256],
                    ps.rearrange("p (t s) -> p t s", t=2),
                    bk4[:, cp : cp + 1],
                )

            def proj_v(St):
                xt, wt = xin["v"], win["v"]
                ps = pp.tile([128, 512], f32, tag="pp", name="psv")
                for D in range(4):
                    nc.tensor.matmul(
                        ps,
                        xt[:, D, 128 * St : 128 * St + 128],
                        wt[:, D, :],
                        start=(D == 0),
                        stop=False,
                    )
                nc.tensor.matmul(
                    ps, ones1[0:1, :], bv1[0:1, :], start=False, stop=True
                )
                t, half = St // 2, St % 2
                for c in range(8):
                    nc.vector.tensor_copy(
                        Vt[t][:, 2 * c + half, 0:64], ps[:, 64 * c : 64 * c + 64]
                    )

            # PE warm-up: ~5us of back-to-back matmuls on the first-loaded
            # weight tile so the HAM un-throttles before the real work.
            wups = pp.tile([128, 512], f32, tag="pp", name="wups")
            for i in range(28):
                nc.tensor.matmul(
                    wups,
                    wup[:, 0:128],
                    wup[:, :],
                    start=(i == 0),
                    stop=(i == 27),
                )

            from collections import deque

            proj_todo = deque()
            # upfront: everything phase (t=0, H=0) and early PVs need
            for cq in range(4):
                proj_q(cq)
            proj_k(0)
            # rest spread into the stream
            proj_todo.append(lambda: proj_v(0))
            proj_todo.append(lambda: proj_k(1))
            proj_todo.append(lambda: proj_v(1))
            for cq in range(4, 8):
                proj_todo.append(lambda cq=cq: proj_q(cq))
            for cp in range(2, 4):
                proj_todo.append(lambda cp=cp: proj_k(cp))
            proj_todo.append(lambda: proj_v(2))
            proj_todo.append(lambda: proj_v(3))

            # ---- attention + merge, 4 phases (head, q'-half), with PV
            # lagged LAG steps behind the S/exp stream so PE never blocks
            # on freshly produced probs ----
            acc = small.tile([128, 16, 512], f32, tag="acc")
            phases = [(t, H) for t in range(2) for H in range(2)]
            O_tiles = {}
            LAG = 6
            pend = deque()

            merge_q = deque()

            def do_merge(pi):
                # Emit the cheap DVE prologue now; queue the 8 per-tile
                # (matmul + normalize [+ dma]) groups to be spread across
                # subsequent stream steps so they don't block the PE FIFO.
                t, H = phases[pi]
                Oa, Ob = O_tiles[pi]
                OT = OTb[pi % 2]
                denrow = drp.tile([1, 1024], f32, tag="dr", name="denrow")
                for n2, Ox in ((0, Oa), (1, Ob)):
                    nc.vector.tensor_copy(
                        denrow[:, 512 * n2 : 512 * n2 + 512], Ox[64:65, :]
                    )
                    nc.vector.tensor_copy(
                        OT[0:64, 512 * n2 : 512 * n2 + 512],
                        Ox[0:64, :],
                    )
                recipT = rcp.tile([128, 8], f32, tag="rc", name="recipT")

                def den_transpose(denrow=denrow, recipT=recipT):
                    denT = pp.tile([128, 8], f32, tag="pp", name="denT")
                    for ii in range(8):
                        nc.tensor.matmul(
                            denT[:, ii : ii + 1],
                            denrow[0:1, 128 * ii : 128 * ii + 128],
                            ones_tiny[0:1, 0:1],
                        )
                    nc.vector.reciprocal(recipT, denT)

                merge_q.append(den_transpose)

                def merge_tile(ii, alt=False, t=t, H=H, OT=OT, recipT=recipT):
                    i = 8 * H + ii
                    mp = pp.tile([128, 512], f32, tag="pp", name="mp")
                    nc.tensor.matmul(mp, OT[:, 128 * ii : 128 * ii + 128], wm_sb[t])
                    blk = (i % 2) * 8 + i // 2
                    if t == 0:
                        nc.vector.tensor_scalar_mul(
                            acc[:, blk, :], mp, recipT[:, ii : ii + 1]
                        )
                    elif alt:
                        # tail-latency path: normalize on ACT, add on GpSimd
                        tmp = tmpp.tile([128, 512], f32, tag="tmp", name="tmp")
                        nc.scalar.activation(
                            tmp, mp, Copy, scale=recipT[:, ii : ii + 1]
                        )
                        nc.gpsimd.tensor_add(acc[:, blk, :], acc[:, blk, :], tmp)
                    else:
                        nc.vector.scalar_tensor_tensor(
                            acc[:, blk, :],
                            mp,
                            recipT[:, ii : ii + 1],
                            acc[:, blk, :],
                            mult,
                            add,
                        )

                def out_dma(b2):
                    # half H merge wrote blk = b2*8 + a for a in H*4..H*4+4
                    # (i = 8H+ii -> blk = (i%2)*8 + i//2); rows 1024*b2+8p+a
                    nc.sync.dma_start(
                        out=d_out_r[:, b2, 4 * H : 4 * H + 4, :],
                        in_=acc[:, 8 * b2 + 4 * H : 8 * b2 + 4 * H + 4, :],
                    )

                if t == 1:
                    last = pi == len(phases) - 1
                    for pos, ii in enumerate((0, 2, 4, 6)):
                        merge_q.append(
                            lambda ii=ii, a=last and ii in (2, 6): merge_tile(ii, a)
                        )
                    merge_q.append(lambda: out_dma(0))
                    for ii in (1, 3, 5, 7):
                        merge_q.append(
                            lambda ii=ii, a=last and ii in (3, 7): merge_tile(ii, a)
                        )
                    merge_q.append(lambda: out_dma(1))
                else:
                    for ii in range(8):
                        merge_q.append(lambda ii=ii: merge_tile(ii))

            def issue_pv(pi, k, pt):
                t, H = phases[pi]
                jp, half = k // 2, k % 2
                jj = jp + 8 * half
                for n in range(2):
                    nc.tensor.matmul(
                        O_tiles[pi][n][0:65, :],
                        Vt[t][:, jj, :],
                        pt[:, 512 * n : 512 * n + 512],
                        start=(k == 0),
                        stop=(k == 15),
                    )
                if k == 15:
                    do_merge(pi)

            for pi, (t, H) in enumerate(phases):
                O_tiles[pi] = (
                    op.tile([128, 512], f32, tag="op", name="oaugA"),
                    op.tile([128, 512], f32, tag="op", name="oaugB"),
                )
                for k in range(16):
                    if proj_todo:
                        proj_todo.popleft()()
                    jp, half = k // 2, k % 2
                    qsrc = QLO if half == 0 else QHI
                    sT = sp.tile([128, 1024], f32, tag="sp", name="sT")
                    for n in range(2):
                        nc.tensor.matmul(
                            sT[:, 512 * n : 512 * n + 512],
                            KT2[:, t, 128 * jp : 128 * jp + 128],
                            qsrc[
                                :,
                                t,
                                1024 * H + 512 * n : 1024 * H + 512 * n + 512,
                            ],
                        )
                    pt = ptp.tile([128, 1024], bf16, tag="pt")
                    nc.scalar.activation(pt, sT, Exp, scale=0.125)
                    pend.append((pi, k, pt))
                    if pend and pend[0][0] < pi:
                        issue_pv(*pend.popleft())
                    if len(pend) > LAG:
                        issue_pv(*pend.popleft())
                    if merge_q:
                        merge_q.popleft()()
            while pend:
                issue_pv(*pend.popleft())
            while merge_q:
                merge_q.popleft()()

    nc.finalize()
    _CACHE["nc"] = nc
    return nc


def _pad128(a):
    out = np.zeros((128, a.shape[1]), a.dtype)
    out[:64] = a
    return np.ascontiguousarray(out)


def _prep_in_maps(q, k, v, Wq, Wk, Wv, Wm, bq, bk, bv):
    import ml_dtypes

    f = np.float32
    b16 = ml_dtypes.bfloat16
    # wqT: each 64-col chunk duplicated -> [512, 1024] (col 128c+64r+d)
    WqT = Wq.T.astype(b16)
    WqT = np.ascontiguousarray(
        np.repeat(WqT.reshape(512, 8, 1, 64), 2, axis=2).reshape(512, 1024)
    )
    # wkT: columns paired (c, c+4) -> col 128*cp + 64*h + d = orig (cp+4h)*64+d
    WkT = Wk.T.astype(b16)
    WkT = np.ascontiguousarray(
        WkT.reshape(512, 2, 4, 64).transpose(0, 2, 1, 3).reshape(512, 512)
    )
    WvT = np.ascontiguousarray(Wv.T.astype(b16))
    WmT = np.ascontiguousarray(Wm.T.astype(b16))  # [e_in, e_out]

    t_ = np.ascontiguousarray(bq.astype(f).reshape(8, 64).T)  # [64, 8]
    bq8 = np.ascontiguousarray(np.vstack([t_, t_]))  # [128, 8]
    # bk4[:, cp] = concat(bk[64cp:64cp+64], bk[64(cp+4):64(cp+4)+64])
    kk = bk.astype(f).reshape(8, 64)
    bk4 = np.ascontiguousarray(np.concatenate([kk[0:4].T, kk[4:8].T], axis=0))
    bv1 = np.ascontiguousarray(bv.astype(b16).reshape(1, 512))

    in_maps = []
    for c in range(NCORES):
        b_, j4 = c // 4, c % 4
        r0 = 512 * j4
        h0 = 2 * j4
        m = {
            "xqT": np.ascontiguousarray(q[b_, r0 : r0 + 512, :].T.astype(b16)),
            "xkT": np.ascontiguousarray(k[b_, r0 : r0 + 512, :].T.astype(b16)),
            "xvT": np.ascontiguousarray(v[b_, r0 : r0 + 512, :].T.astype(b16)),
            "wqT": WqT,
            "wkT": WkT,
            "wvT": WvT,
            "wm0": _pad128(WmT[64 * h0 : 64 * h0 + 64, :]),
            "wm1": _pad128(WmT[64 * h0 + 64 : 64 * h0 + 128, :]),
            "bq8": bq8,
            "bk4": bk4,
            "bv1": bv1,
            "ones1": np.ones((1, 128), b16),
            "wup": np.ones((128, 512), b16) * 0.01,
        }
        in_maps.append(m)
    return in_maps


def _reference_fallback(q, k, v, mask, Wq, Wk, Wv, Wm, bq, bk, bv, bm):
    # Only used if mask is nonzero (spec fills it with zeros).
    f = np.float32
    qh = (q.astype(f) @ Wq.T.astype(f) + bq).reshape(B, 8, S, DHEAD)
    kh = (k.astype(f) @ Wk.T.astype(f) + bk).reshape(B, 8, S, DHEAD)
    vh = (v.astype(f) @ Wv.T.astype(f) + bv).reshape(B, 8, S, DHEAD)
    s = np.einsum("bhqd,bhkd->bhqk", qh, kh) / np.sqrt(np.float32(DHEAD))
    s = np.where(mask, np.float32(-1e9), s)
    s = s - s.max(-1, keepdims=True)
    e = np.exp(s)
    p = e / e.sum(-1, keepdims=True)
    attn = np.einsum("bhqk,bhkd->bhqd", p, vh)
    attn = attn.transpose(0, 2, 1, 3).reshape(B, S, HIDDEN)
    return attn @ Wm.T.astype(f) + bm


def kernel(q, k, v, mask, Wq, Wk, Wv, Wm, bq, bk, bv, bm):
    global LAST_EXEC_NS, LAST_RESULTS
    q, k, v = (np.asarray(a, np.float32) for a in (q, k, v))
    mask = np.asarray(mask)
    Wq, Wk, Wv, Wm = (np.asarray(a, np.float32) for a in (Wq, Wk, Wv, Wm))
    bq, bk, bv, bm = (np.asarray(a, np.float32) for a in (bq, bk, bv, bm))

    if mask.any():
        return _reference_fallback(q, k, v, mask, Wq, Wk, Wv, Wm, bq, bk, bv, bm)

    from concourse.bass_utils import run_bass_kernel_spmd

    nc = _build_nc()
    in_maps = _prep_in_maps(q, k, v, Wq, Wk, Wv, Wm, bq, bk, bv)
    res = run_bass_kernel_spmd(nc, in_maps, list(range(NCORES)))
    LAST_RESULTS = res
    LAST_EXEC_NS = getattr(res, "exec_time_ns", None)

    out = np.zeros((B, S, HIDDEN), np.float32)
    for c in range(NCORES):
        out[c // 4] += res.results[c]["outp"]
    out += bm
    return out
